# revision 22
# baseline (speedup 1.0000x reference)
"""Bass/Trainium2 kernel for nn_EnergyOutputCollector.

Math (per batch row b):
    w[c]      = position_weights.flat[cell_ids[c]]
    surface   = scatter(energy * w) -> [B, 1024]   (cell_ids is a permutation)
    h1 = LN(gelu_tanh(surface @ W1 + b1)) * g1 + bb1
    h2 = LN(gelu_tanh(h1 @ W2 + b2)) * g2 + bb2
    out = h2 @ W3 + b3

Strategy:
  - Data-parallel: batch (16384) split across 8 NeuronCores (2048 each).
  - The scatter + position-weight gather + LN affine params fold into the
    weights on the host in exact fp32:
        W1' = (w[:,None] * W1[cell_ids])          (scatter == row gather of W1)
        W2' = diag(g1) @ W2,  bias2' = bb1 @ W2 + b2
        W3' = diag(g2) @ W3,  bias3' = bb2 @ W3 + b3
  - Device: pure 3-layer MLP in fp16 (PE full rate), fp32 PSUM accumulation,
    gelu on ScalarE straight from PSUM, LayerNorm stats via bn_stats/bn_aggr
    on VectorE, one batched fp16 SBUF->SBUF DMA transpose per layer output
    (contraction dim must sit on partitions for the next matmul).
  - LN's 1/sqrt(var+eps) is computed ENTIRELY on VectorE (quake-style bit
    seed + 2 Newton steps, batched per group) so ScalarE only ever runs
    gelu: its activation table set is loaded once and never swapped
    (gelu and sqrt live in different ACT table sets; each swap costs
    1.3-2.7us and serializes against the PSUM-draining gelus).
  - Groups of 2 m-tiles are software-pipelined layer-major; the emission
    order interleaves the next groups' L1 matmuls and defers each group's
    L3 by one group so the PE never waits on the gelu->LN->transpose chain
    (~6us) between its own layers. Per-engine emission order matches
    dependency-readiness order everywhere (engine FIFOs head-of-line
    block on their first unmet dependency): LN stats/chains right after
    their producing L1/L2 block, out-DMAs on the ACT queue behind their
    ScalarE PSUM->SBUF copies, transposes + e-loads on the SP queue.
  - Weights are loaded in n-chunks (separate tiles => chunk-granular DMA
    deps): SP queue order w1_n0, e(0..1), w1_n1..3, w2, w3 gives the first
    L1 blocks full DMA bandwidth (first matmul ~7us in instead of ~39us,
    with the first two L1 blocks iterating n-chunks outermost to track
    chunk arrivals); the first body's h1 transposes ride the idle ACT
    queue meanwhile.
"""

import numpy as np

import concourse.bass as bass
import concourse.mybir as mybir
import concourse.tile as tile
from concourse import bacc
from concourse.bass_utils import run_bass_kernel_spmd

N_CORES = 8
SURF = 1024
HID = 2048
INTER = 2048
OUT = 768
BATCH = 16384
BC = BATCH // N_CORES          # batch per core
MT = BC // 128                 # m-tiles per core (16)
GROUP = 2                      # m-tiles per layer-major group
EPS = 1e-5

F = mybir.ActivationFunctionType
ALU = mybir.AluOpType
F16 = mybir.dt.float16
F32 = mybir.dt.float32
I32 = mybir.dt.int32

_PROGRAM_CACHE: dict = {}
_LAST_EXEC_NS = None


def _build_program(with_b1: bool, with_b2: bool, with_b3: bool, repeats: int = 1,
                   group: int = GROUP, hp_bufs: int = 8, tp_bufs: int = 8,
                   ep_bufs: int = 2, op_bufs: int = 2, unroll: int = 1,
                   ko: int = 0):
    ngroups = MT // group
    nc = bacc.Bacc(None, target_bir_lowering=False, debug=False)

    e = nc.dram_tensor("e", [SURF, BC], F16, kind="ExternalInput")
    w1 = nc.dram_tensor("w1", [SURF, HID], F16, kind="ExternalInput")
    w2 = nc.dram_tensor("w2", [HID, INTER], F16, kind="ExternalInput")
    w3 = nc.dram_tensor("w3", [INTER, OUT], F16, kind="ExternalInput")
    b1d = nc.dram_tensor("b1", [HID], F16, kind="ExternalInput") if with_b1 else None
    b2d = nc.dram_tensor("b2", [INTER], F16, kind="ExternalInput") if with_b2 else None
    b3d = nc.dram_tensor("b3", [OUT], F32, kind="ExternalInput") if with_b3 else None
    out = nc.dram_tensor("out", [BC, OUT], F32, kind="ExternalOutput")

    # K-on-partitions layouts for matmul operands
    et = e.rearrange("(ko p) b -> p ko b", p=128)      # [128, 8, BC]
    w1t = w1.rearrange("(ko p) n -> p ko n", p=128)    # [128, 8, HID]
    w2t = w2.rearrange("(ko p) n -> p ko n", p=128)    # [128, 16, INTER]
    w3t = w3.rearrange("(ko p) n -> p ko n", p=128)    # [128, 16, OUT]

    with tile.TileContext(nc) as tc:
        with (
            tc.tile_pool(name="weights", bufs=1) as wp,
            tc.tile_pool(name="consts", bufs=1) as cp,
            tc.tile_pool(name="etiles", bufs=ep_bufs) as ep,
            tc.tile_pool(name="acts", bufs=hp_bufs) as hp,
            tc.tile_pool(name="actsT", bufs=tp_bufs) as tp,
            tc.tile_pool(name="stats", bufs=3 * group) as sp,
            tc.tile_pool(name="rstd", bufs=8) as rp,
            tc.tile_pool(name="outs", bufs=op_bufs) as op,
            tc.tile_pool(name="psum", bufs=2, space="PSUM") as pp,
        ):
            # w1 in 4 n-chunks on the SP queue; the first e-group load is
            # emitted between chunk 0 and chunks 1-3 (see _full_body) so L1
            # starts as soon as ~1.5MB has landed.
            w1_sb = [wp.tile([128, 8, 512], F16, tag=f"w1_{n}", name=f"w1_{n}")
                     for n in range(4)]
            nc.sync.dma_start(w1_sb[0][:], w1t[:, :, 0:512])

            def load_w1_rest():
                for n in range(1, 4):
                    nc.sync.dma_start(w1_sb[n][:], w1t[:, :, n * 512:(n + 1) * 512])

            # w2/w3 n-chunks stream on the SP queue AFTER w1/e(0)/e(1) (so the
            # first L1 blocks get the full DMA bandwidth); the first body's
            # h1 transposes go out on the otherwise-idle ACT queue instead.
            w2_sb = [wp.tile([128, 16, 512], F16, tag=f"w2_{n}", name=f"w2_{n}")
                     for n in range(4)]
            w3_sb = [wp.tile([128, 16, 384], F16, tag=f"w3_{n}", name=f"w3_{n}")
                     for n in range(2)]

            def load_w23():
                for n in range(4):
                    nc.sync.dma_start(w2_sb[n][:], w2t[:, :, n * 512:(n + 1) * 512])
                for n in range(2):
                    nc.sync.dma_start(w3_sb[n][:], w3t[:, :, n * 384:(n + 1) * 384])

            def bias_bc(dram_vec, n, tag, dt=F32):
                t = cp.tile([128, n], dt, tag=tag)
                src = bass.AP(
                    tensor=dram_vec.tensor,
                    offset=dram_vec.offset,
                    ap=[[0, 128]] + list(dram_vec.ap),
                )
                nc.scalar.dma_start(t[:], src)
                return t

            b1_sb = bias_bc(b1d[:], HID, "b1", F16) if with_b1 else None
            b2_sb = bias_bc(b2d[:], INTER, "b2", F16) if with_b2 else None
            b3_sb = bias_bc(b3d[:], OUT, "b3") if with_b3 else None

            def tiles_of(g):
                return list(range(g * group, (g + 1) * group))

            def mlp_layer(lhsT_of, w_chunks, nk, bias_sb, tiles, n_outer=False,
                          k_outer=False):
                """psum = lhsT.T @ w (+bias), gelu -> fp16 [128, 2048] tiles.

                n_outer=True iterates weight n-chunks outermost (both tiles'
                psums live) so the first L1 blocks track the chunked w1 DMA
                arrivals without starving the PE.
                """
                hs = {}
                pss = {}
                for t in tiles:
                    hs[t] = hp.tile([128, 2048], F16, tag="h", name="h")
                    pss[t] = pp.tile([128, 2048], F32, tag="ps", name="ps")
                nw = len(w_chunks)
                if k_outer:
                    # k outermost within a tile: runs of `nw` consecutive
                    # matmuls share the same stationary lhsT (fewer exposed
                    # LDWEIGHTS); the psum n-regions accumulate interleaved.
                    loops = [(t, n, k) for t in tiles
                             for k in range(nk) for n in range(nw)]
                elif n_outer:
                    loops = [(t, n, k) for n in range(nw)
                             for t in tiles for k in range(nk)]
                else:
                    loops = [(t, n, k) for t in tiles
                             for n in range(nw) for k in range(nk)]
                for t, n, k in loops:
                    psl = slice(n * 512, (n + 1) * 512)
                    nc.tensor.matmul(
                        pss[t][:, psl], lhsT_of(t, k), w_chunks[n][:, k, :],
                        start=(k == 0), stop=(k == nk - 1),
                    )
                for t in tiles:
                    if bias_sb is not None:
                        nc.vector.tensor_add(
                            out=pss[t][:], in0=pss[t][:], in1=bias_sb[:]
                        )
                    nc.scalar.activation(
                        out=hs[t][:], in_=pss[t][:], func=F.Gelu_apprx_tanh
                    )
                return hs

            def _rsqrt(var_ap, width):
                """1/sqrt(var_ap + eps) on VectorE only: quake-style bit
                seed + 2 Newton steps. Returns the [128, width] result AP."""
                v = rp.tile([128, width], F32, tag=f"v{width}", name="v")
                vh = rp.tile([128, width], F32, tag=f"vh{width}", name="vh")
                yi = rp.tile([128, width], I32, tag=f"yi{width}", name="yi")
                tmp = rp.tile([128, width], F32, tag=f"tmp{width}", name="tmp")
                nc.vector.tensor_scalar_add(out=v[:], in0=var_ap, scalar1=EPS)
                nc.vector.tensor_scalar_mul(out=vh[:], in0=v[:], scalar1=0.5)
                nc.vector.tensor_scalar(
                    out=yi[:], in0=v[:].bitcast(I32), scalar1=1, scalar2=-1,
                    op0=ALU.logical_shift_right, op1=ALU.bitwise_xor,
                )
                nc.vector.tensor_scalar_add(out=yi[:], in0=yi[:], scalar1=0x5F3759E0)
                y = yi[:].bitcast(F32)
                for _ in range(2):
                    nc.vector.tensor_mul(out=tmp[:], in0=y, in1=y)
                    nc.vector.tensor_mul(out=tmp[:], in0=tmp[:], in1=vh[:])
                    nc.vector.tensor_scalar(
                        out=tmp[:], in0=tmp[:], scalar1=-1.0, scalar2=1.5,
                        op0=ALU.mult, op1=ALU.add,
                    )
                    nc.vector.tensor_mul(out=y, in0=y, in1=tmp[:])
                return yi

            def _ln_apply_transpose(h, mean_ap, rstd_ap, act_q=False):
                nc.vector.tensor_scalar(
                    out=h[:], in0=h[:], scalar1=mean_ap, scalar2=rstd_ap,
                    op0=ALU.subtract, op1=ALU.mult,
                )
                ht = tp.tile([128, 16, 128], F16, tag="ht", name="ht")
                eng = nc.scalar if act_q else nc.sync
                eng.dma_start_transpose(ht[:], h[:])
                return ht

            def layernorm_transpose(hs, tiles, per_tile=False, act_q=False):
                """LN (in place) then batched transpose; returns hT tiles.

                per_tile=True runs the rsqrt chain per tile instead of
                batched over the group: slightly more VectorE work, but tile
                i's transpose no longer waits on tile j's stats (used for the
                last group where that wait is the PE tail stall).
                """
                hts = {}
                if per_tile:
                    for t in tiles:
                        stats = sp.tile([128, 4, 6], F32, tag="stats")
                        mv = sp.tile([128, 1, 2], F32, tag="mv")
                        for n in range(4):
                            nc.vector.bn_stats(
                                out=stats[:, n, :],
                                in_=hs[t][:, n * 512:(n + 1) * 512],
                            )
                        nc.vector.bn_aggr(out=mv[:, 0, :], in_=stats[:])
                        yi = _rsqrt(mv[:, :, 1], 1)
                        hts[t] = _ln_apply_transpose(
                            hs[t], mv[:, 0, 0:1], yi[:, 0:1].bitcast(F32),
                            act_q=act_q,
                        )
                    return hts
                mvg = sp.tile([128, group, 2], F32, tag="mvg")
                for i, t in enumerate(tiles):
                    stats = sp.tile([128, 4, 6], F32, tag="stats")
                    for n in range(4):
                        nc.vector.bn_stats(
                            out=stats[:, n, :], in_=hs[t][:, n * 512:(n + 1) * 512]
                        )
                    nc.vector.bn_aggr(out=mvg[:, i, :], in_=stats[:])
                yi = _rsqrt(mvg[:, :, 1], group)
                for i, t in enumerate(tiles):
                    hts[t] = _ln_apply_transpose(
                        hs[t], mvg[:, i, 0:1], yi[:, i:i + 1].bitcast(F32),
                        act_q=act_q,
                    )
                return hts

            def _full_body(first=False):
                e_gs = {}
                ew = group * 128  # batch columns per e-group tile

                def load_e(g):
                    e_sb = ep.tile([128, 8, ew], F16, tag="e")
                    nc.sync.dma_start(
                        e_sb[:], et[:, :, g * ew:(g + 1) * ew]
                    )
                    e_gs[g] = e_sb

                def L1(g, n_outer=False):
                    return mlp_layer(
                        lambda t, k: e_gs[g][:, k, (t % group) * 128:
                                             (t % group + 1) * 128],
                        w1_sb, 8, b1_sb, tiles_of(g), n_outer=n_outer,
                        k_outer=bool(ko) and not n_outer,
                    )

                def L2(g, h1T):
                    return mlp_layer(
                        lambda t, k: h1T[t][:, k, :], w2_sb, 16, b2_sb,
                        tiles_of(g), k_outer=bool(ko),
                    )

                def L3(g, h2T):
                    for t in tiles_of(g):
                        ps = pp.tile([128, 2048], F32, tag="ps", name="ps_l3")
                        l3_loops = (
                            [(n, k) for k in range(16) for n in range(2)]
                            if ko else
                            [(n, k) for n in range(2) for k in range(16)]
                        )
                        for n, k in l3_loops:
                            bsl = slice(n * 512, n * 512 + 384)
                            nc.tensor.matmul(
                                ps[:, bsl], h2T[t][:, k, :],
                                w3_sb[n][:, k, :],
                                start=(k == 0), stop=(k == 15),
                            )
                        ps3 = ps[:, :1024].rearrange(
                            "p (b f) -> p b f", f=512)[:, :2, :384]
                        o_sb = op.tile([128, 2, 384], F32, tag="o")
                        if b3_sb is not None:
                            nc.vector.tensor_tensor(
                                o_sb[:], ps3,
                                b3_sb[:].rearrange("p (b f) -> p b f", f=384),
                                ALU.add,
                            )
                        else:
                            # ScalarE copy (Copy lives in the gelu table set, no
                            # swap); keeps the copy+out-DMA chain off the DVE/SP
                            # queues whose heads the PE critically depends on.
                            nc.scalar.copy(o_sb[:], ps3)
                        nc.scalar.dma_start(
                            out[t * 128:(t + 1) * 128, :],
                            o_sb[:].rearrange("p b f -> p (b f)"),
                        )

                load_e(0)
                if first:
                    # SP queue: w1_n0, e(0), w1_n1..3, e(1), w2, w3, ...
                    # (weights get full DMA bandwidth; the first body's h1
                    # transposes are issued on the idle ACT queue).
                    load_w1_rest()
                load_e(1)
                if first:
                    load_w23()
                h1a = L1(0, n_outer=first)
                h1b = L1(1, n_outer=first)
                h1Ts = {0: layernorm_transpose(h1a, tiles_of(0), act_q=first)}
                h1Ts[1] = layernorm_transpose(h1b, tiles_of(1), act_q=first)
                h2Ts = {}
                for g in range(ngroups):
                    h2 = L2(g, h1Ts.pop(g))
                    if g + 2 < ngroups:
                        load_e(g + 2)
                        h1Ts[g + 2] = layernorm_transpose(
                            L1(g + 2), tiles_of(g + 2))
                    h2Ts[g] = layernorm_transpose(
                        h2, tiles_of(g), per_tile=(g == ngroups - 1))
                    if g >= 1:
                        L3(g - 1, h2Ts.pop(g - 1))
                L3(ngroups - 1, h2Ts.pop(ngroups - 1))

            if repeats == 1:
                _full_body(first=True)
            else:
                _full_body(first=True)
                n_rest = repeats - 1
                assert n_rest % unroll == 0, (repeats, unroll)
                with tc.For_i(0, n_rest // unroll, 1):
                    for _ in range(unroll):
                        _full_body()

    nc.compile()
    return nc


def _prepare(energy, cell_ids, position_weights, W1, b1, ln1_g, ln1_b,
             W2, b2, ln2_g, ln2_b, W3, b3):
    """Host-side prep: shard + fold scatter/gather/LN-affine into weights.
    Returns (bias_flags_key, per-core input maps)."""
    energy = np.asarray(energy, dtype=np.float32)
    cell_ids = np.asarray(cell_ids)
    position_weights = np.asarray(position_weights, dtype=np.float32)
    W1 = np.asarray(W1, dtype=np.float32)
    W2 = np.asarray(W2, dtype=np.float32)
    W3 = np.asarray(W3, dtype=np.float32)
    b1 = np.asarray(b1, dtype=np.float32)
    b2 = np.asarray(b2, dtype=np.float32)
    b3 = np.asarray(b3, dtype=np.float32)
    ln1_g = np.asarray(ln1_g, dtype=np.float32)
    ln1_b = np.asarray(ln1_b, dtype=np.float32)
    ln2_g = np.asarray(ln2_g, dtype=np.float32)
    ln2_b = np.asarray(ln2_b, dtype=np.float32)

    ids = cell_ids.astype(np.int64)
    # scatter surface[:, ids] = (energy * w).T  ==  row-gather of W1 at ids
    # (ids is a permutation: fill=arange per the problem spec)
    w = position_weights.reshape(-1)[ids]
    W1f = w[:, None] * W1[ids]

    # fold LN affine params into the next layer (exact fp32 host math):
    # (xn*g + lb) @ W + b  ==  xn @ (diag(g) W) + (lb @ W + b)
    W2f = ln1_g[:, None] * W2
    b2f = ln1_b @ W2 + b2
    W3f = ln2_g[:, None] * W3
    b3f = ln2_b @ W3 + b3

    with_b1 = bool(np.any(b1 != 0.0))
    with_b2 = bool(np.any(b2f != 0.0))
    with_b3 = bool(np.any(b3f != 0.0))
    key = (with_b1, with_b2, with_b3)

    base = {
        "w1": W1f.astype(np.float16),
        "w2": W2f.astype(np.float16),
        "w3": W3f.astype(np.float16),
    }
    if with_b1:
        base["b1"] = b1.astype(np.float16)
    if with_b2:
        base["b2"] = b2f.astype(np.float16)
    if with_b3:
        base["b3"] = b3f

    e16 = energy.astype(np.float16)
    in_maps = [
        {**base, "e": np.ascontiguousarray(e16[:, c * BC:(c + 1) * BC])}
        for c in range(N_CORES)
    ]
    return key, in_maps


def kernel(energy, cell_ids, position_weights, W1, b1, ln1_g, ln1_b,
           W2, b2, ln2_g, ln2_b, W3, b3):
    key, in_maps = _prepare(energy, cell_ids, position_weights, W1, b1,
                            ln1_g, ln1_b, W2, b2, ln2_g, ln2_b, W3, b3)
    if key not in _PROGRAM_CACHE:
        _PROGRAM_CACHE[key] = _build_program(*key)
    nc = _PROGRAM_CACHE[key]
    res = run_bass_kernel_spmd(nc, in_maps, core_ids=list(range(N_CORES)))
    global _LAST_EXEC_NS
    if res.exec_time_ns is not None:
        _LAST_EXEC_NS = res.exec_time_ns
    return np.concatenate([r["out"] for r in res.results], axis=0)
